# revision 9
# baseline (speedup 1.0000x reference)
"""Trainium2 Bass kernel for nn_RSHN (heterogeneous GNN message passing).

Contract: kernel(**inputs) takes the FULL unsharded inputs (as produced by the
problem's setup_inputs) and returns the FULL [N_A+N_B, 128] float32 output.

Strategy (8 NeuronCores, SPMD):
  - Shard by destination node: core c owns rows [c*N/8, (c+1)*N/8) of each node
    type. Host buckets each relation's edges by (dst core, src half, dst block
    of 128), sorts them, and pads each block's list to a multiple of 128.
  - On device, f[src] rows are fetched with dma_gather (fp16 tables, 256B
    rows, 4 SWDGE queues), one-hot(dst_local) tiles are built on DVE with a
    broadcast is_equal against an iota tile, and segment-sum becomes PE
    matmuls accumulating aggT[feat, dst_block] in PSUM per relation.
  - The per-relation edge weight w_r (from the tiny on-device AGNN relation
    encoder) is folded into the dense layer weight: W'_r = diag(w_r) @ W, so
    each 128-row dst block finishes with 3 dense matmuls + tanh.
  - One AllGather per node type between the two GraphConv layers rebuilds the
    replicated node tables; the final W_out projection is applied per block.
"""
import math

import numpy as np

NCORES = 8
P = 128
NQ = 4  # SWDGE queues used round-robin for gathers

_CACHE: dict = {}
_SIM = False  # set True by tests to run in MultiCoreSim instead of HW
_TRACE = False  # set True by tests to capture an NTFF profile
_LAST_RESULT = None  # BassKernelResults of the last HW run (for tests)


# ----------------------------------------------------------------------------
# Host preprocessing
# ----------------------------------------------------------------------------

def _prow(n, sh, ss):
    return (n // sh) * ss + (n % sh)


def _prep_edges(src, dst, sh_src, ss_src, half_rows, sh_dst, nblk_dst, drop_self):
    """Bucket one relation's edges per (core, half, block).

    Returns (NCH[half][blk] uniform chunk counts, per-core streams):
      streams[c][half] = (idx16 array [nch_tot*128], dstloc array [nch_tot*128])
    """
    src = np.asarray(src, np.int64)
    dst = np.asarray(dst, np.int64)
    if drop_self:
        keep = src != dst
        src, dst = src[keep], dst[keep]
    psrc = _prow(src, sh_src, ss_src)
    half = (psrc >= half_rows).astype(np.int64)
    idx16 = (psrc - half * half_rows).astype(np.int16)
    core = dst // sh_dst
    loc = dst % sh_dst
    blk = loc // P
    dloc = (loc % P).astype(np.int16)

    order = np.lexsort((blk, half, core))
    idx16, dloc = idx16[order], dloc[order]
    key = (core[order] * 2 + half[order]) * nblk_dst + blk[order]
    nkeys = NCORES * 2 * nblk_dst
    counts = np.bincount(key, minlength=nkeys).reshape(NCORES, 2, nblk_dst)
    starts = np.zeros(nkeys + 1, np.int64)
    np.cumsum(counts.reshape(-1), out=starts[1:])

    nch = np.ceil(counts / P).astype(np.int64).max(axis=0)  # [2, nblk]
    choff = np.zeros((2, nblk_dst), np.int64)
    tot = [0, 0]
    for h in range(2):
        for b in range(nblk_dst):
            choff[h, b] = tot[h]
            tot[h] += nch[h, b]

    streams = []
    for c in range(NCORES):
        per_half = []
        for h in range(2):
            si = np.zeros(tot[h] * P, np.int16)
            sd = np.full(tot[h] * P, -1, np.int16)
            for b in range(nblk_dst):
                k = (c * 2 + h) * nblk_dst + b
                n = counts[c, h, b]
                if n == 0:
                    continue
                o = choff[h, b] * P
                si[o : o + n] = idx16[starts[k] : starts[k] + n]
                sd[o : o + n] = dloc[starts[k] : starts[k] + n]
            per_half.append((si, sd))
        streams.append(per_half)
    return nch, choff, tot, streams


def _wrap_idx(si):
    """[n*128] int16 -> [128, n*8] wrapped (j -> [j%16, j//16]) + replicated."""
    if si.size == 0:
        return np.zeros((P, 0), np.int16)
    a = si.reshape(-1, 16).T  # [16, n*8]
    return np.tile(a, (8, 1)).copy()


def _dst_t(sd):
    """[n*128] int16 -> [128, n] fp16 transposed (value of edge p of chunk c
    at [p, c])."""
    if sd.size == 0:
        return np.zeros((P, 0), np.float16)
    return sd.reshape(-1, P).T.astype(np.float16).copy()


def _pad_table(f, sh, ss, dtype=np.float16):
    """[N, D] -> [8*ss, D] with per-core stride ss (tail rows zero)."""
    n, d = f.shape
    out = np.zeros((NCORES * ss, d), dtype)
    for c in range(NCORES):
        out[c * ss : c * ss + sh] = f[c * sh : (c + 1) * sh]
    return out


def _onehot(idx, n):
    out = np.zeros((len(idx), n), np.float32)
    out[np.arange(len(idx)), np.asarray(idx)] = 1.0
    return out


# ----------------------------------------------------------------------------
# Program builder
# ----------------------------------------------------------------------------

def _build(struct):
    import concourse.bacc as bacc
    import concourse.mybir as mybir
    import concourse.tile as tile

    D = struct["D"]
    sh_a, ss_a, nblk_a = struct["sh_a"], struct["ss_a"], struct["nblk_a"]
    sh_b, ss_b, nblk_b = struct["sh_b"], struct["ss_b"], struct["nblk_b"]
    trows_a, trows_b = NCORES * ss_a, NCORES * ss_b
    gmax = struct["gmax"]
    NCH = struct["NCH"]  # {rel: [2, nblk]}
    CHOFF = struct["CHOFF"]
    TOT = struct["TOT"]  # {rel: [tot_lo, tot_hi]}
    f16, f32, i16 = mybir.dt.float16, mybir.dt.float32, mybir.dt.int16
    AF = mybir.ActivationFunctionType
    OP = mybir.AluOpType

    nc = bacc.Bacc(
        "TRN2",
        target_bir_lowering=False,
        debug=False,
        num_devices=NCORES,
        num_swdge_queues=NQ,
    )

    # --- inputs ---
    din = {}

    def inp(name, shape, dt):
        din[name] = nc.dram_tensor(name, list(shape), dt, kind="ExternalInput")
        return din[name]

    fA0 = inp("fA0", [trows_a, D], f16)
    fB0 = inp("fB0", [trows_b, D], f16)
    fA0o = inp("fA0o", [ss_a, D], f16)
    fB0o = inp("fB0o", [ss_b, D], f16)
    idx_in, dst_in = {}, {}
    for rel in ("AA", "BA", "AB"):
        for h in range(2):
            t = TOT[rel][h]
            idx_in[(rel, h)] = inp(f"idx_{rel}_{h}", [P, max(t, 1) * 8], i16)
            dst_in[(rel, h)] = inp(f"dst_{rel}_{h}", [P, max(t, 1)], f16)
    iota_in = inp("iota", [P, gmax * P], f16)
    clh_in = inp("clh", [3, D], f32)
    clw_in = inp("clw", [1, 6], f32)
    b1_in = inp("b1", [1, 1], f32)
    b2_in = inp("b2", [1, 1], f32)
    We1_in = inp("We1", [D, D], f32)
    be1_in = inp("be1", [1, D], f32)
    Wg1_in = inp("Wg1", [D, D], f32)
    Wg2_in = inp("Wg2", [D, D], f32)
    Wout_in = inp("Wout", [D, D], f32)
    S_in = inp("S6x3", [6, 3], f32)
    ST_in = inp("ST3x6", [3, 6], f32)
    Doh_in = inp("Doh6x3", [6, 3], f32)
    id3_in = inp("id3", [3, 3], f32)
    ones3_in = inp("ones3", [1, 3], f32)

    out_d = nc.dram_tensor("out", [sh_a + sh_b, D], f32, kind="ExternalOutput")

    with tile.TileContext(nc) as tc:
        with (
            tc.tile_pool(name="const", bufs=1) as cp,
            tc.tile_pool(name="dram", bufs=1, space="DRAM") as dp,
            tc.tile_pool(name="gath", bufs=3) as gp,
            tc.tile_pool(name="oh", bufs=3) as op_,
            tc.tile_pool(name="sb", bufs=3) as sp,
            tc.tile_pool(name="enc", bufs=2) as ep,
            tc.tile_pool(name="ps", bufs=2, space="PSUM") as pp,
        ):
            # DRAM intermediates
            fA1 = dp.tile([trows_a, D], f16, addr_space="Shared")
            fB1 = dp.tile([trows_b, D], f16, addr_space="Shared")
            fA1c = dp.tile([ss_a, D], f16)
            fB1c = dp.tile([ss_b, D], f16)

            # --- load constants ---
            def load(t, src):
                tt = cp.tile(list(src.shape), src.dtype, tag=str(t), name=str(t))
                nc.sync.dma_start(tt[:], src[:])
                return tt

            iota_sb = load("iota", iota_in)
            idx_sb = {k: load(f"idx{k}", v) for k, v in idx_in.items()}
            dst_sb = {k: load(f"dst{k}", v) for k, v in dst_in.items()}
            clw_sb = load("clw", clw_in)
            b1_sb = load("b1", b1_in)
            b2_sb = load("b2", b2_in)
            We1_sb = load("We1", We1_in)
            be1_sb = load("be1", be1_in)
            Wg1_sb = load("Wg1", Wg1_in)
            Wg2_sb = load("Wg2", Wg2_in)
            Wout_sb = load("Wout", Wout_in)
            S_sb = load("S", S_in)
            ST_sb = load("ST", ST_in)
            Doh_sb = load("Doh", Doh_in)
            id3_sb = load("id3", id3_in)
            ones3_sb = load("ones3", ones3_in)

            # =============== relation encoder (tiny, fp32) ===============
            h = cp.tile([3, D], f32, name="h0", tag="h0")
            nc.sync.dma_start(h[:], clh_in[:])
            for bt_in in (b1_sb, b2_sb):
                e_ps = pp.tile([6, 1], f32, space="PSUM", tag="pf")
                nc.tensor.matmul(e_ps[:], lhsT=clw_sb[:], rhs=bt_in[:],
                                 start=True, stop=True)
                pe = ep.tile([6, 1], f32, tag="e1")
                nc.scalar.activation(pe[:], e_ps[:], AF.Exp)
                den_ps = pp.tile([3, 1], f32, space="PSUM", tag="pf")
                nc.tensor.matmul(den_ps[:], lhsT=S_sb[:], rhs=pe[:],
                                 start=True, stop=True)
                den = ep.tile([3, 1], f32, tag="e2")
                nc.vector.tensor_copy(den[:], den_ps[:])
                dpe_ps = pp.tile([6, 1], f32, space="PSUM", tag="pf")
                nc.tensor.matmul(dpe_ps[:], lhsT=ST_sb[:], rhs=den[:],
                                 start=True, stop=True)
                dpe = ep.tile([6, 1], f32, tag="e3")
                nc.vector.tensor_copy(dpe[:], dpe_ps[:])
                rec = ep.tile([6, 1], f32, tag="e4")
                nc.vector.reciprocal(rec[:], dpe[:])
                pr = ep.tile([6, 1], f32, tag="e5")
                nc.vector.tensor_tensor(out=pr[:], in0=pe[:], in1=rec[:],
                                        op=OP.mult)
                sq = ep.tile([3, D], f32, tag="e6")
                nc.vector.tensor_tensor(out=sq[:], in0=h[:], in1=h[:],
                                        op=OP.mult)
                ss_ = ep.tile([3, 1], f32, tag="e7")
                nc.vector.tensor_reduce(ss_[:], sq[:],
                                        axis=mybir.AxisListType.X, op=OP.add)
                nrm = ep.tile([3, 1], f32, tag="e8")
                nc.scalar.activation(nrm[:], ss_[:], AF.Sqrt)
                nrmc = ep.tile([3, 1], f32, tag="e9")
                nc.vector.tensor_scalar_max(nrmc[:], nrm[:], 1e-12)
                rn = ep.tile([3, 1], f32, tag="e10")
                nc.vector.reciprocal(rn[:], nrmc[:])
                nh = ep.tile([3, D], f32, tag="e11")
                nc.vector.tensor_scalar_mul(nh[:], h[:], rn[:])
                nhs_ps = pp.tile([6, D], f32, space="PSUM", tag="pf")
                nc.tensor.matmul(nhs_ps[:], lhsT=ST_sb[:], rhs=nh[:],
                                 start=True, stop=True)
                nhs = ep.tile([6, D], f32, tag="e12")
                nc.vector.tensor_copy(nhs[:], nhs_ps[:])
                sc = ep.tile([6, D], f32, tag="e13")
                nc.vector.tensor_scalar_mul(sc[:], nhs[:], pr[:])
                agg_ps = pp.tile([3, D], f32, space="PSUM", tag="pf")
                nc.tensor.matmul(agg_ps[:], lhsT=Doh_sb[:], rhs=sc[:],
                                 start=True, stop=True)
                hsum = ep.tile([3, D], f32, tag="e14")
                nc.vector.tensor_tensor(out=hsum[:], in0=h[:], in1=agg_ps[:],
                                        op=OP.add)
                h2 = cp.tile([3, D], f32, name=f"h_{id(bt_in)}", tag=f"h_{id(bt_in)}")
                nc.scalar.activation(h2[:], hsum[:], AF.Relu)
                h = h2
            hT_ps = pp.tile([D, 3], f32, space="PSUM", tag="pf")
            nc.tensor.matmul(hT_ps[:], lhsT=h[:], rhs=id3_sb[:],
                             is_transpose=True, start=True, stop=True)
            hT = ep.tile([D, 3], f32, tag="e15")
            nc.vector.tensor_copy(hT[:], hT_ps[:])
            hrT_ps = pp.tile([D, 3], f32, space="PSUM", tag="pf")
            nc.tensor.matmul(hrT_ps[:], lhsT=We1_sb[:], rhs=hT[:],
                             start=True, stop=False)
            nc.tensor.matmul(hrT_ps[:], lhsT=be1_sb[:], rhs=ones3_sb[:],
                             start=False, stop=True)
            hrT = cp.tile([D, 3], f32, name="hrT", tag="hrT")
            nc.vector.tensor_copy(hrT[:], hrT_ps[:])

            # folded weights (fp16): W'_r = diag(w_r) @ W ; col 0=AA, 1=AB, 2=BA
            Wp = {}
            for li, wsb in ((1, Wg1_sb), (2, Wg2_sb)):
                for rel, col in (("AA", 0), ("AB", 1), ("BA", 2)):
                    t = cp.tile([D, D], f16, name=f"Wp{li}{rel}", tag=f"Wp{li}{rel}")
                    nc.vector.tensor_scalar_mul(t[:], wsb[:], hrT[:, col : col + 1])
                    Wp[(li, rel)] = t
            W16 = {}
            for li, wsb in ((1, Wg1_sb), (2, Wg2_sb)):
                t = cp.tile([D, D], f16, name=f"W16_{li}", tag=f"W16_{li}")
                nc.vector.tensor_copy(t[:], wsb[:])
                W16[li] = t
            Wout16 = cp.tile([D, D], f16, name="Wout16", tag="Wout16")
            nc.vector.tensor_copy(Wout16[:], Wout_sb[:])

            # =============== graph conv layers ===============
            qctr = [0]

            def layer(li, tabA, tabB, ownA, ownB, halfrow_a, halfrow_b):
                for X, streams, rels, nblk, own, sh_x in (
                    ("A", [("AA", 0), ("AA", 1), ("BA", 0), ("BA", 1)],
                     ["AA", "BA"], nblk_a, ownA, sh_a),
                    ("B", [("AB", 0), ("AB", 1)], ["AB"], nblk_b, ownB, sh_b),
                ):
                    srctab = {"AA": tabA, "BA": tabB, "AB": tabA}
                    halfrows = {"AA": halfrow_a, "BA": halfrow_b, "AB": halfrow_a}
                    for b in range(nblk):
                        gts, ohs = {}, {}
                        for si_, (rel, hh) in enumerate(streams):
                            nch = int(NCH[rel][hh][b])
                            if nch == 0:
                                continue
                            off = int(CHOFF[rel][hh][b])
                            tab = srctab[rel]
                            hr = halfrows[rel]
                            base = tab[hr * hh : hr * (hh + 1), :] if hh == 0 else tab[hr:, :]
                            gt = gp.tile([P, nch, D], f16, tag=f"g{si_}",
                                         name=f"g{li}{X}{b}{si_}")
                            nc.gpsimd.dma_gather(
                                out_ap=gt[:],
                                in_ap=base,
                                idxs_ap=idx_sb[(rel, hh)][:, off * 8 : (off + nch) * 8],
                                num_idxs=nch * P,
                                num_idxs_reg=nch * P,
                                elem_size=D,
                                single_packet=False,
                                queue_num=qctr[0] % NQ,
                            )
                            qctr[0] += 1
                            oh = op_.tile([P, nch, P], f16, tag=f"o{si_}",
                                          name=f"o{li}{X}{b}{si_}")
                            nc.vector.tensor_tensor(
                                out=oh[:],
                                in0=iota_sb[:, : nch * P].rearrange(
                                    "p (c d) -> p c d", d=P),
                                in1=dst_sb[(rel, hh)][:, off : off + nch]
                                .to_broadcast([P, nch, P]),
                                op=OP.is_equal,
                            )
                            gts[(rel, hh)] = gt
                            ohs[(rel, hh)] = oh
                        # per-relation aggT accumulation
                        aggs = {}
                        for ri, rel in enumerate(rels):
                            ktot = sum(
                                int(NCH[rel][hh][b]) for hh in range(2))
                            if ktot == 0:
                                continue
                            ps = pp.tile([P, P], f32, space="PSUM",
                                         tag=f"agg{ri}", name=f"ps{li}{X}{b}{ri}")
                            k = 0
                            for hh in range(2):
                                nch = int(NCH[rel][hh][b])
                                for j in range(nch):
                                    nc.tensor.matmul(
                                        ps[:],
                                        lhsT=gts[(rel, hh)][:, j, :],
                                        rhs=ohs[(rel, hh)][:, j, :],
                                        start=(k == 0),
                                        stop=(k == ktot - 1),
                                    )
                                    k += 1
                            a16 = sp.tile([P, P], f16, tag=f"a16_{ri}",
                                          name=f"a{li}{X}{b}{ri}")
                            nc.vector.tensor_copy(a16[:], ps[:])
                            aggs[rel] = a16
                        # residual rows, transposed [feat, dst]
                        ft = sp.tile([P, P], f16, tag="ft", name=f"ft{li}{X}{b}")
                        nc.sync.dma_start(
                            ft[:], own[b * P : (b + 1) * P, :], transpose=True)
                        # combine
                        pf = pp.tile([P, P], f32, space="PSUM", tag="pf",
                                     name=f"pf{li}{X}{b}")
                        if li == 1:
                            mlist = [(aggs[rel], Wp[(1, rel)]) for rel in aggs]
                            mlist.append((ft, W16[1]))
                        else:
                            mlist = [(Wp[(2, rel)], aggs[rel]) for rel in aggs]
                            mlist.append((W16[2], ft))
                        for i, (lh, rh) in enumerate(mlist):
                            nc.tensor.matmul(pf[:], lhsT=lh[:], rhs=rh[:],
                                             start=(i == 0),
                                             stop=(i == len(mlist) - 1))
                        rows = P if b < nblk - 1 else (sh_x - P * (nblk - 1))
                        if li == 1:
                            f1t = sp.tile([P, P], f16, tag="f1t",
                                          name=f"f1t{X}{b}")
                            nc.scalar.activation(f1t[:], pf[:], AF.Tanh)
                            contrib = fA1c if X == "A" else fB1c
                            nc.sync.dma_start(
                                contrib[b * P : b * P + rows, :], f1t[:rows, :])
                        else:
                            f2t = sp.tile([P, P], f16, tag="f2t",
                                          name=f"f2t{X}{b}")
                            nc.scalar.activation(f2t[:], pf[:], AF.Tanh)
                            po = pp.tile([P, P], f32, space="PSUM", tag="po",
                                         name=f"po{X}{b}")
                            nc.tensor.matmul(po[:], lhsT=f2t[:], rhs=Wout16[:],
                                             start=True, stop=True)
                            oc = sp.tile([P, P], f32, tag="oc", name=f"oc{X}{b}")
                            nc.vector.tensor_copy(oc[:], po[:])
                            xoff = 0 if X == "A" else sh_a
                            nc.sync.dma_start(
                                out_d[xoff + b * P : xoff + b * P + rows, :],
                                oc[:rows, :])

            layer(1, fA0, fB0, fA0o, fB0o, trows_a // 2, trows_b // 2)
            # allgather layer-1 features
            nc.gpsimd.collective_compute(
                "AllGather", mybir.AluOpType.bypass,
                replica_groups=[list(range(NCORES))],
                ins=[fA1c.opt()], outs=[fA1.opt()],
            )
            nc.gpsimd.collective_compute(
                "AllGather", mybir.AluOpType.bypass,
                replica_groups=[list(range(NCORES))],
                ins=[fB1c.opt()], outs=[fB1.opt()],
            )
            layer(2, fA1, fB1, fA1c, fB1c, trows_a // 2, trows_b // 2)

    nc.compile()
    # Align each gather's SWDGE queue with its Tile-assigned DMASW sem lane:
    # a sem lane may only be incremented from one SWDGE queue, and Tile's
    # round-robin lane assignment ignores queue_num. lane k -> queue k % NQ
    # keeps every lane single-queue while still spreading work over queues.
    for bb in nc.main_func.blocks:
        for ins in bb.instructions:
            if type(ins).__name__ == "InstDMAGatherAnt":
                si = ins.sync_info
                if si is not None and si.on_update:
                    name = si.on_update[0].ant_name or ""
                    if name.startswith("DMASW"):
                        lane = int(name[5:].split("_")[0])
                        ins.queue_num = lane % NQ
    return nc


# ----------------------------------------------------------------------------
# kernel entry
# ----------------------------------------------------------------------------

def kernel(**inputs):
    inputs = {k: np.asarray(v) for k, v in inputs.items()}
    feat_A, feat_B = inputs["feat_A"], inputs["feat_B"]
    n_a, D = feat_A.shape
    n_b = feat_B.shape[0]
    assert n_a % NCORES == 0 and n_b % NCORES == 0 and D == 128
    sh_a, sh_b = n_a // NCORES, n_b // NCORES
    ss_a = (sh_a + P - 1) // P * P
    ss_b = (sh_b + P - 1) // P * P
    nblk_a, nblk_b = ss_a // P, ss_b // P
    half_a, half_b = NCORES * ss_a // 2, NCORES * ss_b // 2
    assert half_a <= 32768 and half_b <= 32768

    rel_defs = {
        "AA": (inputs["src_AA"], inputs["dst_AA"], sh_a, ss_a, half_a, sh_a, nblk_a, True),
        "BA": (inputs["src_BA"], inputs["dst_BA"], sh_b, ss_b, half_b, sh_a, nblk_a, False),
        "AB": (inputs["src_AB"], inputs["dst_AB"], sh_a, ss_a, half_a, sh_b, nblk_b, False),
    }
    NCH, CHOFF, TOT, STREAMS = {}, {}, {}, {}
    for rel, (s, d, shs, sss, hrows, shd, nblkd, drop) in rel_defs.items():
        nch, choff, tot, streams = _prep_edges(s, d, shs, sss, hrows, shd, nblkd, drop)
        NCH[rel], CHOFF[rel], TOT[rel], STREAMS[rel] = nch, choff, tot, streams

    gmax = max(1, max(int(NCH[rel].max()) for rel in NCH))

    struct_key = (
        n_a, n_b, gmax,
        tuple((rel, NCH[rel].tobytes()) for rel in sorted(NCH)),
    )
    if struct_key in _CACHE:
        nc, runner = _CACHE[struct_key]
    else:
        struct = dict(D=D, sh_a=sh_a, ss_a=ss_a, nblk_a=nblk_a,
                      sh_b=sh_b, ss_b=ss_b, nblk_b=nblk_b,
                      gmax=gmax, NCH=NCH, CHOFF=CHOFF, TOT=TOT)
        nc = _build(struct)
        runner = None
        _CACHE.clear()
        _CACHE[struct_key] = (nc, runner)

    # per-core input maps
    fA0p = _pad_table(feat_A.astype(np.float16), sh_a, ss_a)
    fB0p = _pad_table(feat_B.astype(np.float16), sh_b, ss_b)
    iota = np.tile(np.arange(P, dtype=np.float16), (P, gmax)).reshape(P, gmax * P)

    cl_src = np.asarray(inputs["cl_src"], np.int64)
    cl_dst = np.asarray(inputs["cl_dst"], np.int64)
    common = {
        "fA0": fA0p, "fB0": fB0p, "iota": iota,
        "clh": inputs["cl_h"].astype(np.float32),
        "clw": inputs["cl_w"].astype(np.float32).reshape(1, 6),
        "b1": inputs["beta1"].astype(np.float32).reshape(1, 1),
        "b2": inputs["beta2"].astype(np.float32).reshape(1, 1),
        "We1": inputs["W_e1"].astype(np.float32),
        "be1": inputs["b_e1"].astype(np.float32).reshape(1, D),
        "Wg1": inputs["Wg1"].astype(np.float32),
        "Wg2": inputs["Wg2"].astype(np.float32),
        "Wout": inputs["W_out"].astype(np.float32),
        "S6x3": _onehot(cl_src, 3),
        "ST3x6": _onehot(cl_src, 3).T.copy(),
        "Doh6x3": _onehot(cl_dst, 3),
        "id3": np.eye(3, dtype=np.float32),
        "ones3": np.ones((1, 3), np.float32),
    }
    in_maps = []
    for c in range(NCORES):
        m = dict(common)
        m["fA0o"] = fA0p[c * ss_a : (c + 1) * ss_a].copy()
        m["fB0o"] = fB0p[c * ss_b : (c + 1) * ss_b].copy()
        for rel in ("AA", "BA", "AB"):
            for h in range(2):
                si, sd = STREAMS[rel][c][h]
                iw, dt_ = _wrap_idx(si), _dst_t(sd)
                if iw.shape[1] == 0:
                    iw = np.zeros((P, 8), np.int16)
                    dt_ = np.zeros((P, 1), np.float16)
                m[f"idx_{rel}_{h}"] = iw
                m[f"dst_{rel}_{h}"] = dt_
        in_maps.append(m)

    if _SIM:
        from concourse.bass_interp import MultiCoreSim

        sim = MultiCoreSim(nc, num_cores=NCORES, require_finite=False,
                           require_nnan=False)
        for c in range(NCORES):
            for k, v in in_maps[c].items():
                sim.cores[c].tensor(k)[:] = v
        sim.simulate(check_with_hw=False)
        res = [{"out": np.array(sim.cores[c].tensor("out"))} for c in range(NCORES)]
    else:
        from concourse import bass_utils

        global _LAST_RESULT
        r = bass_utils.run_bass_kernel_spmd(
            nc, in_maps, core_ids=list(range(NCORES)), trace=_TRACE
        )
        _LAST_RESULT = r
        res = r.results

    out = np.empty((n_a + n_b, D), np.float32)
    for c in range(NCORES):
        o = res[c]["out"]
        out[c * sh_a : (c + 1) * sh_a] = o[:sh_a]
        out[n_a + c * sh_b : n_a + (c + 1) * sh_b] = o[sh_a : sh_a + sh_b]
    return out


# revision 12
# speedup vs baseline: 1.0297x; 1.0297x over previous
"""Trainium2 Bass kernel for nn_RSHN (heterogeneous GNN message passing).

Contract: kernel(**inputs) takes the FULL unsharded inputs (as produced by the
problem's setup_inputs) and returns the FULL [N_A+N_B, 128] float32 output.

Strategy (8 NeuronCores, SPMD):
  - Shard by destination node: core c owns rows [c*N/8, (c+1)*N/8) of each node
    type. Host buckets each relation's edges by (dst core, src half, dst block
    of 128), sorts them, and pads each block's list to a multiple of 128.
  - On device, f[src] rows are fetched with dma_gather (fp16 tables, 256B
    rows, 4 SWDGE queues), one-hot(dst_local) tiles are built on DVE with a
    broadcast is_equal against an iota tile, and segment-sum becomes PE
    matmuls accumulating aggT[feat, dst_block] in PSUM per relation.
  - The per-relation edge weight w_r (from the tiny on-device AGNN relation
    encoder) is folded into the dense layer weight: W'_r = diag(w_r) @ W, so
    each 128-row dst block finishes with 3 dense matmuls + tanh.
  - One AllGather per node type between the two GraphConv layers rebuilds the
    replicated node tables; the final W_out projection is applied per block.
"""
import math

import numpy as np

NCORES = 8
P = 128
NQ = 4  # SWDGE queues used round-robin for gathers

_CACHE: dict = {}
_SIM = False  # set True by tests to run in MultiCoreSim instead of HW
_TRACE = False  # set True by tests to capture an NTFF profile
_LAST_RESULT = None  # BassKernelResults of the last HW run (for tests)


# ----------------------------------------------------------------------------
# Host preprocessing
# ----------------------------------------------------------------------------

def _prow(n, sh, ss):
    return (n // sh) * ss + (n % sh)


def _prep_edges(src, dst, sh_src, ss_src, half_rows, sh_dst, nblk_dst, drop_self):
    """Bucket one relation's edges per (core, half, block).

    Returns (NCH[half][blk] uniform chunk counts, per-core streams):
      streams[c][half] = (idx16 array [nch_tot*128], dstloc array [nch_tot*128])
    """
    src = np.asarray(src, np.int64)
    dst = np.asarray(dst, np.int64)
    if drop_self:
        keep = src != dst
        src, dst = src[keep], dst[keep]
    psrc = _prow(src, sh_src, ss_src)
    half = (psrc >= half_rows).astype(np.int64)
    idx16 = (psrc - half * half_rows).astype(np.int16)
    core = dst // sh_dst
    loc = dst % sh_dst
    blk = loc // P
    dloc = (loc % P).astype(np.int16)

    order = np.lexsort((blk, half, core))
    idx16, dloc = idx16[order], dloc[order]
    key = (core[order] * 2 + half[order]) * nblk_dst + blk[order]
    nkeys = NCORES * 2 * nblk_dst
    counts = np.bincount(key, minlength=nkeys).reshape(NCORES, 2, nblk_dst)
    starts = np.zeros(nkeys + 1, np.int64)
    np.cumsum(counts.reshape(-1), out=starts[1:])

    nch = np.ceil(counts / P).astype(np.int64).max(axis=0)  # [2, nblk]
    choff = np.zeros((2, nblk_dst), np.int64)
    tot = [0, 0]
    for h in range(2):
        for b in range(nblk_dst):
            choff[h, b] = tot[h]
            tot[h] += nch[h, b]

    streams = []
    for c in range(NCORES):
        per_half = []
        for h in range(2):
            si = np.zeros(tot[h] * P, np.int16)
            sd = np.full(tot[h] * P, -1, np.int16)
            for b in range(nblk_dst):
                k = (c * 2 + h) * nblk_dst + b
                n = counts[c, h, b]
                if n == 0:
                    continue
                o = choff[h, b] * P
                si[o : o + n] = idx16[starts[k] : starts[k] + n]
                sd[o : o + n] = dloc[starts[k] : starts[k] + n]
            per_half.append((si, sd))
        streams.append(per_half)
    return nch, choff, tot, streams


def _wrap_idx(si):
    """[n*128] int16 -> [128, n*8] wrapped (j -> [j%16, j//16]) + replicated."""
    if si.size == 0:
        return np.zeros((P, 0), np.int16)
    a = si.reshape(-1, 16).T  # [16, n*8]
    return np.tile(a, (8, 1)).copy()


def _dst_t(sd):
    """[n*128] int16 -> [128, n] fp16 transposed (value of edge p of chunk c
    at [p, c])."""
    if sd.size == 0:
        return np.zeros((P, 0), np.float16)
    return sd.reshape(-1, P).T.astype(np.float16).copy()


def _pad_table(f, sh, ss, dtype=np.float16):
    """[N, D] -> [8*ss, D] with per-core stride ss (tail rows zero)."""
    n, d = f.shape
    out = np.zeros((NCORES * ss, d), dtype)
    for c in range(NCORES):
        out[c * ss : c * ss + sh] = f[c * sh : (c + 1) * sh]
    return out


def _onehot(idx, n):
    out = np.zeros((len(idx), n), np.float32)
    out[np.arange(len(idx)), np.asarray(idx)] = 1.0
    return out


# ----------------------------------------------------------------------------
# Program builder
# ----------------------------------------------------------------------------

def _build(struct):
    import concourse.bacc as bacc
    import concourse.mybir as mybir
    import concourse.tile as tile

    D = struct["D"]
    sh_a, ss_a, nblk_a = struct["sh_a"], struct["ss_a"], struct["nblk_a"]
    sh_b, ss_b, nblk_b = struct["sh_b"], struct["ss_b"], struct["nblk_b"]
    trows_a, trows_b = NCORES * ss_a, NCORES * ss_b
    gmax = struct["gmax"]
    NCH = struct["NCH"]  # {rel: [2, nblk]}
    CHOFF = struct["CHOFF"]
    TOT = struct["TOT"]  # {rel: [tot_lo, tot_hi]}
    f16, f32, i16 = mybir.dt.float16, mybir.dt.float32, mybir.dt.int16
    AF = mybir.ActivationFunctionType
    OP = mybir.AluOpType

    nc = bacc.Bacc(
        "TRN2",
        target_bir_lowering=False,
        debug=False,
        num_devices=NCORES,
        num_swdge_queues=NQ,
    )

    # --- inputs ---
    din = {}

    def inp(name, shape, dt):
        din[name] = nc.dram_tensor(name, list(shape), dt, kind="ExternalInput")
        return din[name]

    fA0 = inp("fA0", [trows_a, D], f16)
    fB0 = inp("fB0", [trows_b, D], f16)
    fA0o = inp("fA0o", [ss_a, D], f16)
    fB0o = inp("fB0o", [ss_b, D], f16)
    idx_in, dst_in = {}, {}
    for rel in ("AA", "BA", "AB"):
        for h in range(2):
            t = TOT[rel][h]
            idx_in[(rel, h)] = inp(f"idx_{rel}_{h}", [P, max(t, 1) * 8], i16)
            dst_in[(rel, h)] = inp(f"dst_{rel}_{h}", [P, max(t, 1)], f16)
    iota_in = inp("iota", [P, gmax * P], f16)
    clh_in = inp("clh", [3, D], f32)
    clw_in = inp("clw", [1, 6], f32)
    b1_in = inp("b1", [1, 1], f32)
    b2_in = inp("b2", [1, 1], f32)
    We1_in = inp("We1", [D, D], f32)
    be1_in = inp("be1", [1, D], f32)
    Wg1_in = inp("Wg1", [D, D], f32)
    Wg2_in = inp("Wg2", [D, D], f32)
    Wout_in = inp("Wout", [D, D], f32)
    S_in = inp("S6x3", [6, 3], f32)
    ST_in = inp("ST3x6", [3, 6], f32)
    Doh_in = inp("Doh6x3", [6, 3], f32)
    id3_in = inp("id3", [3, 3], f32)
    ones3_in = inp("ones3", [1, 3], f32)

    out_d = nc.dram_tensor("out", [sh_a + sh_b, D], f32, kind="ExternalOutput")

    with tile.TileContext(nc) as tc:
        with (
            tc.tile_pool(name="const", bufs=1) as cp,
            tc.tile_pool(name="dram", bufs=1, space="DRAM") as dp,
            tc.tile_pool(name="gath", bufs=2) as gp,
            tc.tile_pool(name="oh", bufs=2) as op_,
            tc.tile_pool(name="sb", bufs=3) as sp,
            tc.tile_pool(name="enc", bufs=2) as ep,
            tc.tile_pool(name="ps", bufs=2, space="PSUM") as pp,
        ):
            # DRAM intermediates
            fA1 = dp.tile([trows_a, D], f16, addr_space="Shared")
            fB1 = dp.tile([trows_b, D], f16, addr_space="Shared")
            fA1c = dp.tile([ss_a, D], f16)
            fB1c = dp.tile([ss_b, D], f16)

            # --- load constants ---
            def load(t, src):
                tt = cp.tile(list(src.shape), src.dtype, tag=str(t), name=str(t))
                nc.sync.dma_start(tt[:], src[:])
                return tt

            iota_sb = load("iota", iota_in)
            idx_sb = {k: load(f"idx{k}", v) for k, v in idx_in.items()}
            dst_sb = {k: load(f"dst{k}", v) for k, v in dst_in.items()}
            clw_sb = load("clw", clw_in)
            b1_sb = load("b1", b1_in)
            b2_sb = load("b2", b2_in)
            We1_sb = load("We1", We1_in)
            be1_sb = load("be1", be1_in)
            Wg1_sb = load("Wg1", Wg1_in)
            Wg2_sb = load("Wg2", Wg2_in)
            Wout_sb = load("Wout", Wout_in)
            S_sb = load("S", S_in)
            ST_sb = load("ST", ST_in)
            Doh_sb = load("Doh", Doh_in)
            id3_sb = load("id3", id3_in)
            ones3_sb = load("ones3", ones3_in)

            # =============== relation encoder (tiny, fp32) ===============
            h = cp.tile([3, D], f32, name="h0", tag="h0")
            nc.sync.dma_start(h[:], clh_in[:])
            for bt_in in (b1_sb, b2_sb):
                e_ps = pp.tile([6, 1], f32, space="PSUM", tag="pf")
                nc.tensor.matmul(e_ps[:], lhsT=clw_sb[:], rhs=bt_in[:],
                                 start=True, stop=True)
                pe = ep.tile([6, 1], f32, tag="e1")
                nc.scalar.activation(pe[:], e_ps[:], AF.Exp)
                den_ps = pp.tile([3, 1], f32, space="PSUM", tag="pf")
                nc.tensor.matmul(den_ps[:], lhsT=S_sb[:], rhs=pe[:],
                                 start=True, stop=True)
                den = ep.tile([3, 1], f32, tag="e2")
                nc.vector.tensor_copy(den[:], den_ps[:])
                dpe_ps = pp.tile([6, 1], f32, space="PSUM", tag="pf")
                nc.tensor.matmul(dpe_ps[:], lhsT=ST_sb[:], rhs=den[:],
                                 start=True, stop=True)
                dpe = ep.tile([6, 1], f32, tag="e3")
                nc.vector.tensor_copy(dpe[:], dpe_ps[:])
                rec = ep.tile([6, 1], f32, tag="e4")
                nc.vector.reciprocal(rec[:], dpe[:])
                pr = ep.tile([6, 1], f32, tag="e5")
                nc.vector.tensor_tensor(out=pr[:], in0=pe[:], in1=rec[:],
                                        op=OP.mult)
                sq = ep.tile([3, D], f32, tag="e6")
                nc.vector.tensor_tensor(out=sq[:], in0=h[:], in1=h[:],
                                        op=OP.mult)
                ss_ = ep.tile([3, 1], f32, tag="e7")
                nc.vector.tensor_reduce(ss_[:], sq[:],
                                        axis=mybir.AxisListType.X, op=OP.add)
                nrm = ep.tile([3, 1], f32, tag="e8")
                nc.scalar.activation(nrm[:], ss_[:], AF.Sqrt)
                nrmc = ep.tile([3, 1], f32, tag="e9")
                nc.vector.tensor_scalar_max(nrmc[:], nrm[:], 1e-12)
                rn = ep.tile([3, 1], f32, tag="e10")
                nc.vector.reciprocal(rn[:], nrmc[:])
                nh = ep.tile([3, D], f32, tag="e11")
                nc.vector.tensor_scalar_mul(nh[:], h[:], rn[:])
                nhs_ps = pp.tile([6, D], f32, space="PSUM", tag="pf")
                nc.tensor.matmul(nhs_ps[:], lhsT=ST_sb[:], rhs=nh[:],
                                 start=True, stop=True)
                nhs = ep.tile([6, D], f32, tag="e12")
                nc.vector.tensor_copy(nhs[:], nhs_ps[:])
                sc = ep.tile([6, D], f32, tag="e13")
                nc.vector.tensor_scalar_mul(sc[:], nhs[:], pr[:])
                agg_ps = pp.tile([3, D], f32, space="PSUM", tag="pf")
                nc.tensor.matmul(agg_ps[:], lhsT=Doh_sb[:], rhs=sc[:],
                                 start=True, stop=True)
                hsum = ep.tile([3, D], f32, tag="e14")
                nc.vector.tensor_tensor(out=hsum[:], in0=h[:], in1=agg_ps[:],
                                        op=OP.add)
                h2 = cp.tile([3, D], f32, name=f"h_{id(bt_in)}", tag=f"h_{id(bt_in)}")
                nc.scalar.activation(h2[:], hsum[:], AF.Relu)
                h = h2
            hT_ps = pp.tile([D, 3], f32, space="PSUM", tag="pf")
            nc.tensor.matmul(hT_ps[:], lhsT=h[:], rhs=id3_sb[:],
                             is_transpose=True, start=True, stop=True)
            hT = ep.tile([D, 3], f32, tag="e15")
            nc.vector.tensor_copy(hT[:], hT_ps[:])
            hrT_ps = pp.tile([D, 3], f32, space="PSUM", tag="pf")
            nc.tensor.matmul(hrT_ps[:], lhsT=We1_sb[:], rhs=hT[:],
                             start=True, stop=False)
            nc.tensor.matmul(hrT_ps[:], lhsT=be1_sb[:], rhs=ones3_sb[:],
                             start=False, stop=True)
            hrT = cp.tile([D, 3], f32, name="hrT", tag="hrT")
            nc.vector.tensor_copy(hrT[:], hrT_ps[:])

            # folded weights (fp16): W'_r = diag(w_r) @ W ; col 0=AA, 1=AB, 2=BA
            Wp = {}
            for li, wsb in ((1, Wg1_sb), (2, Wg2_sb)):
                for rel, col in (("AA", 0), ("AB", 1), ("BA", 2)):
                    t = cp.tile([D, D], f16, name=f"Wp{li}{rel}", tag=f"Wp{li}{rel}")
                    nc.vector.tensor_scalar_mul(t[:], wsb[:], hrT[:, col : col + 1])
                    Wp[(li, rel)] = t
            W16 = {}
            for li, wsb in ((1, Wg1_sb), (2, Wg2_sb)):
                t = cp.tile([D, D], f16, name=f"W16_{li}", tag=f"W16_{li}")
                nc.vector.tensor_copy(t[:], wsb[:])
                W16[li] = t
            Wout16 = cp.tile([D, D], f16, name="Wout16", tag="Wout16")
            nc.vector.tensor_copy(Wout16[:], Wout_sb[:])

            # =============== graph conv layers ===============
            qctr = [0]
            GB = struct["gblk"]  # dst blocks per gather group

            def phase(li, X, tabA, tabB, own):
                streams = ([("AA", 0), ("AA", 1), ("BA", 0), ("BA", 1)]
                           if X == "A" else [("AB", 0), ("AB", 1)])
                rels = ["AA", "BA"] if X == "A" else ["AB"]
                nblk = nblk_a if X == "A" else nblk_b
                sh_x = sh_a if X == "A" else sh_b
                srctab = {"AA": tabA, "BA": tabB, "AB": tabA}
                halfrows = {"AA": trows_a // 2, "BA": trows_b // 2,
                            "AB": trows_a // 2}
                for g0 in range(0, nblk, GB):
                    g1 = min(g0 + GB, nblk)
                    gts, ohs, goffs = {}, {}, {}
                    for si_, (rel, hh) in enumerate(streams):
                        goff = int(CHOFF[rel][hh][g0])
                        nq = sum(int(NCH[rel][hh][b]) for b in range(g0, g1))
                        if nq == 0:
                            continue
                        goffs[(rel, hh)] = goff
                        hr = halfrows[rel]
                        base = srctab[rel][hr * hh : hr * (hh + 1), :]
                        gt = gp.tile([P, nq, D], f16, tag=f"g{si_}",
                                     name=f"g{li}{X}{g0}_{si_}")
                        nc.gpsimd.dma_gather(
                            out_ap=gt[:],
                            in_ap=base,
                            idxs_ap=idx_sb[(rel, hh)][:, goff * 8 : (goff + nq) * 8],
                            num_idxs=nq * P,
                            num_idxs_reg=nq * P,
                            elem_size=D,
                            single_packet=False,
                            queue_num=qctr[0] % NQ,
                        )
                        qctr[0] += 1
                        oh = op_.tile([P, nq, P], f16, tag=f"o{si_}",
                                      name=f"o{li}{X}{g0}_{si_}")
                        nc.vector.tensor_tensor(
                            out=oh[:],
                            in0=iota_sb[:, : nq * P].rearrange(
                                "p (c d) -> p c d", d=P),
                            in1=dst_sb[(rel, hh)][:, goff : goff + nq]
                            .to_broadcast([P, nq, P]),
                            op=OP.is_equal,
                        )
                        gts[(rel, hh)] = gt
                        ohs[(rel, hh)] = oh
                    for b in range(g0, g1):
                        # per-relation aggT accumulation
                        aggs = {}
                        for ri, rel in enumerate(rels):
                            ktot = sum(int(NCH[rel][hh][b]) for hh in range(2))
                            if ktot == 0:
                                continue
                            ps = pp.tile([P, P], f32, space="PSUM",
                                         tag=f"agg{ri}", name=f"ps{li}{X}{b}{ri}")
                            k = 0
                            for hh in range(2):
                                nch = int(NCH[rel][hh][b])
                                if nch == 0:
                                    continue
                                lo = int(CHOFF[rel][hh][b]) - goffs[(rel, hh)]
                                for j in range(nch):
                                    nc.tensor.matmul(
                                        ps[:],
                                        lhsT=gts[(rel, hh)][:, lo + j, :],
                                        rhs=ohs[(rel, hh)][:, lo + j, :],
                                        start=(k == 0),
                                        stop=(k == ktot - 1),
                                    )
                                    k += 1
                            a16 = sp.tile([P, P], f16, tag=f"a16_{ri}",
                                          name=f"a{li}{X}{b}{ri}")
                            nc.vector.tensor_copy(a16[:], ps[:])
                            aggs[rel] = a16
                        # residual rows, transposed [feat, dst]
                        ft = sp.tile([P, P], f16, tag="ft", name=f"ft{li}{X}{b}")
                        nc.sync.dma_start(
                            ft[:], own[b * P : (b + 1) * P, :], transpose=True)
                        # combine
                        pf = pp.tile([P, P], f32, space="PSUM", tag="pf",
                                     name=f"pf{li}{X}{b}")
                        if li == 1:
                            mlist = [(aggs[rel], Wp[(1, rel)]) for rel in aggs]
                            mlist.append((ft, W16[1]))
                        else:
                            mlist = [(Wp[(2, rel)], aggs[rel]) for rel in aggs]
                            mlist.append((W16[2], ft))
                        for i, (lh, rh) in enumerate(mlist):
                            nc.tensor.matmul(pf[:], lhsT=lh[:], rhs=rh[:],
                                             start=(i == 0),
                                             stop=(i == len(mlist) - 1))
                        rows = P if b < nblk - 1 else (sh_x - P * (nblk - 1))
                        if li == 1:
                            f1t = sp.tile([P, P], f16, tag="f1t",
                                          name=f"f1t{X}{b}")
                            nc.scalar.activation(f1t[:], pf[:], AF.Tanh)
                            contrib = fA1c if X == "A" else fB1c
                            nc.sync.dma_start(
                                contrib[b * P : b * P + rows, :], f1t[:rows, :])
                        else:
                            f2t = sp.tile([P, P], f16, tag="f2t",
                                          name=f"f2t{X}{b}")
                            nc.scalar.activation(f2t[:], pf[:], AF.Tanh)
                            po = pp.tile([P, P], f32, space="PSUM", tag="po",
                                         name=f"po{X}{b}")
                            nc.tensor.matmul(po[:], lhsT=f2t[:], rhs=Wout16[:],
                                             start=True, stop=True)
                            oc = sp.tile([P, P], f32, tag="oc", name=f"oc{X}{b}")
                            nc.vector.tensor_copy(oc[:], po[:])
                            xoff = 0 if X == "A" else sh_a
                            nc.sync.dma_start(
                                out_d[xoff + b * P : xoff + b * P + rows, :],
                                oc[:rows, :])

            # phase order lets each AllGather overlap compute: AG(A) runs
            # during layer-1 B blocks; AG(B) runs during layer-2 B blocks
            # (which only need fA1).
            phase(1, "A", fA0, fB0, fA0o)
            nc.gpsimd.collective_compute(
                "AllGather", mybir.AluOpType.bypass,
                replica_groups=[list(range(NCORES))],
                ins=[fA1c.opt()], outs=[fA1.opt()],
            )
            phase(1, "B", fA0, fB0, fB0o)
            nc.gpsimd.collective_compute(
                "AllGather", mybir.AluOpType.bypass,
                replica_groups=[list(range(NCORES))],
                ins=[fB1c.opt()], outs=[fB1.opt()],
            )
            phase(2, "B", fA1, fB1, fB1c)
            phase(2, "A", fA1, fB1, fA1c)

    nc.compile()
    # Align each gather's SWDGE queue with its Tile-assigned DMASW sem lane:
    # a sem lane may only be incremented from one SWDGE queue, and Tile's
    # round-robin lane assignment ignores queue_num. lane k -> queue k % NQ
    # keeps every lane single-queue while still spreading work over queues.
    for bb in nc.main_func.blocks:
        for ins in bb.instructions:
            if type(ins).__name__ == "InstDMAGatherAnt":
                si = ins.sync_info
                if si is not None and si.on_update:
                    name = si.on_update[0].ant_name or ""
                    if name.startswith("DMASW"):
                        lane = int(name[5:].split("_")[0])
                        ins.queue_num = lane % NQ
    return nc


# ----------------------------------------------------------------------------
# kernel entry
# ----------------------------------------------------------------------------

def kernel(**inputs):
    inputs = {k: np.asarray(v) for k, v in inputs.items()}
    feat_A, feat_B = inputs["feat_A"], inputs["feat_B"]
    n_a, D = feat_A.shape
    n_b = feat_B.shape[0]
    assert n_a % NCORES == 0 and n_b % NCORES == 0 and D == 128
    sh_a, sh_b = n_a // NCORES, n_b // NCORES
    ss_a = (sh_a + P - 1) // P * P
    ss_b = (sh_b + P - 1) // P * P
    nblk_a, nblk_b = ss_a // P, ss_b // P
    half_a, half_b = NCORES * ss_a // 2, NCORES * ss_b // 2
    assert half_a <= 32768 and half_b <= 32768

    rel_defs = {
        "AA": (inputs["src_AA"], inputs["dst_AA"], sh_a, ss_a, half_a, sh_a, nblk_a, True),
        "BA": (inputs["src_BA"], inputs["dst_BA"], sh_b, ss_b, half_b, sh_a, nblk_a, False),
        "AB": (inputs["src_AB"], inputs["dst_AB"], sh_a, ss_a, half_a, sh_b, nblk_b, False),
    }
    NCH, CHOFF, TOT, STREAMS = {}, {}, {}, {}
    for rel, (s, d, shs, sss, hrows, shd, nblkd, drop) in rel_defs.items():
        nch, choff, tot, streams = _prep_edges(s, d, shs, sss, hrows, shd, nblkd, drop)
        NCH[rel], CHOFF[rel], TOT[rel], STREAMS[rel] = nch, choff, tot, streams

    # group size: how many dst blocks share one dma_gather call. Bigger
    # groups amortize the ~2.5us SWDGE descriptor-gen cost per call; bounded
    # by SBUF (gather + one-hot slabs are sized to the max group chunk sum).
    def group_max(gb):
        mx = 1
        for rel in NCH:
            for h in range(2):
                arr = NCH[rel][h]
                for g0_ in range(0, len(arr), gb):
                    mx = max(mx, int(arr[g0_ : g0_ + gb].sum()))
        return mx

    idx_bytes = sum(int(TOT[rel][h]) * 16 for rel in TOT for h in range(2))
    dst_bytes = sum(int(TOT[rel][h]) * 2 for rel in TOT for h in range(2))
    fixed = idx_bytes + dst_bytes + 24 * 1024
    gblk, gmax = 1, group_max(1)
    for gb in (12, 10, 8, 6, 4, 3, 2, 1):
        mx = group_max(gb)
        if fixed + mx * 256 + 16 * mx * 256 <= 170 * 1024:
            gblk, gmax = gb, mx
            break

    struct_key = (
        n_a, n_b, gmax, gblk,
        tuple((rel, NCH[rel].tobytes()) for rel in sorted(NCH)),
    )
    if struct_key in _CACHE:
        nc, runner = _CACHE[struct_key]
    else:
        struct = dict(D=D, sh_a=sh_a, ss_a=ss_a, nblk_a=nblk_a,
                      sh_b=sh_b, ss_b=ss_b, nblk_b=nblk_b,
                      gmax=gmax, gblk=gblk, NCH=NCH, CHOFF=CHOFF, TOT=TOT)
        nc = _build(struct)
        runner = None
        _CACHE.clear()
        _CACHE[struct_key] = (nc, runner)

    # per-core input maps
    fA0p = _pad_table(feat_A.astype(np.float16), sh_a, ss_a)
    fB0p = _pad_table(feat_B.astype(np.float16), sh_b, ss_b)
    iota = np.tile(np.arange(P, dtype=np.float16), (P, gmax)).reshape(P, gmax * P)

    cl_src = np.asarray(inputs["cl_src"], np.int64)
    cl_dst = np.asarray(inputs["cl_dst"], np.int64)
    common = {
        "fA0": fA0p, "fB0": fB0p, "iota": iota,
        "clh": inputs["cl_h"].astype(np.float32),
        "clw": inputs["cl_w"].astype(np.float32).reshape(1, 6),
        "b1": inputs["beta1"].astype(np.float32).reshape(1, 1),
        "b2": inputs["beta2"].astype(np.float32).reshape(1, 1),
        "We1": inputs["W_e1"].astype(np.float32),
        "be1": inputs["b_e1"].astype(np.float32).reshape(1, D),
        "Wg1": inputs["Wg1"].astype(np.float32),
        "Wg2": inputs["Wg2"].astype(np.float32),
        "Wout": inputs["W_out"].astype(np.float32),
        "S6x3": _onehot(cl_src, 3),
        "ST3x6": _onehot(cl_src, 3).T.copy(),
        "Doh6x3": _onehot(cl_dst, 3),
        "id3": np.eye(3, dtype=np.float32),
        "ones3": np.ones((1, 3), np.float32),
    }
    in_maps = []
    for c in range(NCORES):
        m = dict(common)
        m["fA0o"] = fA0p[c * ss_a : (c + 1) * ss_a].copy()
        m["fB0o"] = fB0p[c * ss_b : (c + 1) * ss_b].copy()
        for rel in ("AA", "BA", "AB"):
            for h in range(2):
                si, sd = STREAMS[rel][c][h]
                iw, dt_ = _wrap_idx(si), _dst_t(sd)
                if iw.shape[1] == 0:
                    iw = np.zeros((P, 8), np.int16)
                    dt_ = np.zeros((P, 1), np.float16)
                m[f"idx_{rel}_{h}"] = iw
                m[f"dst_{rel}_{h}"] = dt_
        in_maps.append(m)

    if _SIM:
        from concourse.bass_interp import MultiCoreSim

        sim = MultiCoreSim(nc, num_cores=NCORES, require_finite=False,
                           require_nnan=False)
        for c in range(NCORES):
            for k, v in in_maps[c].items():
                sim.cores[c].tensor(k)[:] = v
        sim.simulate(check_with_hw=False)
        res = [{"out": np.array(sim.cores[c].tensor("out"))} for c in range(NCORES)]
    else:
        from concourse import bass_utils

        global _LAST_RESULT
        r = bass_utils.run_bass_kernel_spmd(
            nc, in_maps, core_ids=list(range(NCORES)), trace=_TRACE
        )
        _LAST_RESULT = r
        res = r.results

    out = np.empty((n_a + n_b, D), np.float32)
    for c in range(NCORES):
        o = res[c]["out"]
        out[c * sh_a : (c + 1) * sh_a] = o[:sh_a]
        out[n_a + c * sh_b : n_a + (c + 1) * sh_b] = o[sh_a : sh_a + sh_b]
    return out
